# revision 20
# baseline (speedup 1.0000x reference)
"""CrystalGraphEncoder Trainium2 kernel (8 NeuronCores, SPMD).

Sharding: nodes split into 8 contiguous row-slices; each core owns edges whose
destination row falls in its slice (sorted by local row). Per layer, each core
computes per-node tables [V|M|U] = x @ [Wg2|Wl|Wg1] + biases for its slice,
the V|M part is AllGathered into a full replicated [N,128] table, then the
edge phase gathers VM[col] and U[row] per 128-edge tile (indirect DMA),
computes msg = sigmoid(U+V) * M, and segment-sums via a one-hot selection
matmul into PSUM per 128-row window. x <- relu(agg + M). Mean-pool partials
are returned per core; the tiny MLP head runs on host.
"""
import sys
import os

sys.path.insert(0, "/opt/trn_rl_repo")

import numpy as np

import concourse.bass as bass
import concourse.mybir as mybir
import concourse.tile as tile
from concourse import bacc
from concourse import bass_utils

# bass_utils imports antenv.axon_hooks when BASS_TRACE is set; provide a
# graceful stub if the image's antenv lacks that module.
try:
    import antenv.axon_hooks  # noqa: F401
except Exception:
    import types as _types
    import antenv as _antenv

    _hooks = _types.ModuleType("antenv.axon_hooks")
    _hooks._HOOK = None
    _hooks.set_axon_ntff_profile_hook = lambda h: setattr(_hooks, "_HOOK", h)
    _hooks.get_axon_ntff_profile_hook = lambda: _hooks._HOOK
    sys.modules["antenv.axon_hooks"] = _hooks
    _antenv.axon_hooks = _hooks

F32 = mybir.dt.float32
BF16 = mybir.dt.bfloat16
I32 = mybir.dt.int32

N_ATOMS = int(os.environ.get("GNN_N", 100000))
N_EDGES = 1000000
H = 64
OUT_DIM = 128
N_LAYERS = 3
NCORES = 8
S = N_ATOMS // NCORES          # 12500 rows per core
NW = (S + 127) // 128          # 98 windows per core
SPAD = NW * 128                # 12544 padded rows
LASTW = S - (NW - 1) * 128     # 84 valid rows in final window
PADCOL = N_ATOMS               # zero row of the VM table


def _prep(inputs):
    """Host-side prep: embedding lookup, per-core edge scheduling, weights."""
    x0 = np.asarray(inputs["emb_table"], np.float32)[np.asarray(inputs["atomic_numbers"])]
    edge = np.asarray(inputs["edge_index"])
    row = edge[0].astype(np.int64)
    col = edge[1].astype(np.int64)

    core_of = row // S
    percore = []
    cntmax = np.zeros(NW, np.int64)
    for c in range(NCORES):
        m = core_of == c
        lr = (row[m] - c * S).astype(np.int32)
        cc = col[m].astype(np.int32)
        o = np.argsort(lr, kind="stable")
        lr, cc = lr[o], cc[o]
        w = lr >> 7
        cnt = np.bincount(w, minlength=NW)
        cntmax = np.maximum(cntmax, cnt)
        percore.append((lr, cc, w, cnt))
    nts = np.maximum(np.ceil(cntmax / 128).astype(np.int64), 1)
    nt_u = int(nts.max())

    # Build uniform [NW, 128, nt_u] (flattened to [SPAD, nt_u*3]) idx arrays.
    packs = []
    for c in range(NCORES):
        lr, cc, w, cnt = percore[c]
        colidx = np.full((NW, nt_u * 128), PADCOL, np.int32)
        uidx = np.zeros((NW, nt_u * 128), np.int32)
        rrel = np.zeros((NW, nt_u * 128), np.float32)
        start = 0
        for wi in range(NW):
            n = int(cnt[wi])
            sl = slice(start, start + n)
            colidx[wi, :n] = cc[sl]
            uidx[wi, :n] = lr[sl]
            rrel[wi, :n] = (lr[sl] & 127).astype(np.float32)
            start += n
        # slot e = t*128 + p  ->  [NW, nt_u, 128] -> [NW, 128, nt_u]
        colidx = colidx.reshape(NW, nt_u, 128).transpose(0, 2, 1)
        uidx = uidx.reshape(NW, nt_u, 128).transpose(0, 2, 1)
        rrel = rrel.reshape(NW, nt_u, 128).transpose(0, 2, 1)
        pack = np.empty((NW, 128, nt_u, 3), np.int32)
        pack[..., 0] = colidx
        pack[..., 1] = uidx
        pack[..., 2] = rrel.view(np.int32)
        packs.append(pack.reshape(SPAD, nt_u * 3))

    # weights: wcat [L, 65, 192] = rows 0..63 [Wg2|Wl|Wg1], row 64 biases [0|bl|bg]
    Wg = np.asarray(inputs["W_gate"], np.float32)   # [L, 2H, H]
    Wl = np.asarray(inputs["W_lin"], np.float32)    # [L, H, H]
    bl = np.asarray(inputs["b_lin"], np.float32)    # [L, H]
    bg = np.asarray(inputs["b_gate"], np.float32)   # [L, H]
    wcat = np.zeros((N_LAYERS, 65, 3 * H), np.float32)
    for l in range(N_LAYERS):
        wcat[l, :H, 0:H] = Wg[l, H:]       # V = x @ Wg2
        wcat[l, :H, H:2 * H] = Wl[l]       # M = x @ Wl
        wcat[l, :H, 2 * H:] = Wg[l, :H]    # U = x @ Wg1
        wcat[l, 64, H:2 * H] = bl[l]
        wcat[l, 64, 2 * H:] = bg[l]

    # x0T slices [64, SPAD] f32, zero-padded
    x0T = []
    for c in range(NCORES):
        xs = np.zeros((H, SPAD), np.float32)
        xs[:, :S] = x0[c * S:(c + 1) * S].T
        x0T.append(np.ascontiguousarray(xs))

    # pooling mask [128, NW] f32
    pm = np.ones((128, NW), np.float32)
    pm[LASTW:, NW - 1] = 0.0

    # iota row replicated [128, 128] f32
    iot = np.broadcast_to(np.arange(128, dtype=np.float32), (128, 128)).copy()

    return packs, wcat, x0T, pm, iot, nt_u, [int(v) for v in nts]


def _build(nt_u, nts):
    nc = bacc.Bacc("TRN2", target_bir_lowering=False, debug=False,
                   num_devices=NCORES)
    pack_d = nc.dram_tensor("pack", [SPAD, nt_u * 3], I32, kind="ExternalInput")
    wcat_d = nc.dram_tensor("wcat", [N_LAYERS, 65, 3 * H], F32, kind="ExternalInput")
    x0t_d = nc.dram_tensor("x0t", [H, SPAD], F32, kind="ExternalInput")
    pm_d = nc.dram_tensor("pm", [128, NW], F32, kind="ExternalInput")
    iota_d = nc.dram_tensor("iota", [128, 128], F32, kind="ExternalInput")
    pooled_d = nc.dram_tensor("pooled", [1, H], F32, kind="ExternalOutput")

    vm_slice = nc.dram_tensor("vm_slice", [S, 2 * H], BF16, kind="Internal")
    vm_full = nc.dram_tensor("vm_full", [N_ATOMS + 1, 2 * H], BF16,
                             kind="Internal", addr_space="Shared")
    u_slice = nc.dram_tensor("u_slice", [S, H], F32, kind="Internal")

    with tile.TileContext(nc) as tc:
        with (
            tc.tile_pool(name="persist", bufs=1) as persist,
            tc.tile_pool(name="stage", bufs=1) as stage,
            tc.tile_pool(name="idxw", bufs=4) as idxw,
            tc.tile_pool(name="edge", bufs=8) as edge,
            tc.tile_pool(name="small", bufs=2) as small,
            tc.tile_pool(name="pnode", bufs=1, space="PSUM") as pnode,
            tc.tile_pool(name="pwin", bufs=3, space="PSUM") as pwin,
            tc.tile_pool(name="ptr", bufs=1, space="PSUM") as ptr,
            tc.tile_pool(name="ppool", bufs=1, space="PSUM") as ppool,
            tc.tile_pool(name="pst", bufs=1, space="PSUM") as pst,
            tc.tile_pool(name="puex", bufs=1, space="PSUM") as puex,
        ):
            xTb = persist.tile([65, SPAD], F32)       # row 64 = ones
            uvm = stage.tile([128, NW * 3 * H], F32)  # [V|M|U] per chunk
            xnew = stage.tile([128, NW * H], F32)
            vmcast = stage.tile([128, NW * 2 * H], BF16)
            ucast = stage.tile([128, NW * H], BF16)
            iot = persist.tile([128, 128], F32)
            pm = persist.tile([128, NW], F32)
            wct = persist.tile([65, N_LAYERS * 3 * H], F32)
            ident = persist.tile([128, 128], F32)
            identb = persist.tile([128, 128], BF16)

            nc.sync.dma_start(out=iot[:], in_=iota_d[:, :])
            nc.sync.dma_start(out=pm[:], in_=pm_d[:, :])
            nc.sync.dma_start(
                out=wct[:].rearrange("p (l f) -> p l f", l=N_LAYERS),
                in_=wcat_d[:, :, :].rearrange("l p f -> p l f"),
            )
            nc.sync.dma_start(out=xTb[0:H, :], in_=x0t_d[:, :])
            nc.vector.memset(xTb[64:65, :], 1.0)
            # identity for PE transpose: ident[p, j] = (j == p)
            iotp = persist.tile([128, 1], F32)
            nc.gpsimd.iota(iotp[:], pattern=[[1, 1]], base=0, channel_multiplier=1,
                           allow_small_or_imprecise_dtypes=True)
            nc.vector.tensor_scalar(
                out=ident[:], in0=iot[:], scalar1=iotp[:, 0:1], scalar2=None,
                op0=mybir.AluOpType.is_equal,
            )
            nc.vector.tensor_copy(out=identb[:], in_=ident[:])
            # zero the pad row of vm_full once
            zrow = small.tile([1, 2 * H], BF16)
            nc.vector.memset(zrow[:], 0.0)
            nc.sync.dma_start(out=vm_full[N_ATOMS:N_ATOMS + 1, :], in_=zrow[:])

            for l in range(N_LAYERS):
                # ---- node phase: [V|M|U] = xTb.T @ wcat[l] per 128-row chunk
                for cch in range(NW):
                    ps = pnode.tile([128, 3 * H], F32, space="PSUM")
                    nc.tensor.matmul(
                        ps[:],
                        lhsT=xTb[:, cch * 128:(cch + 1) * 128],
                        rhs=wct[:, l * 3 * H:(l + 1) * 3 * H],
                        start=True, stop=True,
                    )
                    nc.scalar.copy(
                        out=uvm[:, cch * 3 * H:(cch + 1) * 3 * H], in_=ps[:]
                    )
                # table writes (rows < S only), cast to bf16 first
                vm_ap = uvm[:].rearrange("p (c f) -> p c f", c=NW)[:, :, 0:2 * H]
                vmc_ap = vmcast[:].rearrange("p (c f) -> p c f", c=NW)
                nc.vector.tensor_copy(out=vmc_ap[:, :, :], in_=vm_ap)
                nc.sync.dma_start(
                    out=vm_slice[0:(NW - 1) * 128, :].rearrange(
                        "(c p) f -> p c f", p=128),
                    in_=vmc_ap[:, 0:NW - 1, :],
                )
                nc.sync.dma_start(
                    out=vm_slice[(NW - 1) * 128:S, :],
                    in_=vmc_ap[0:LASTW, NW - 1, :],
                )
                u_ap = uvm[:].rearrange("p (c f) -> p c f", c=NW)[:, :, 2 * H:3 * H]
                nc.vector.tensor_copy(
                    out=ucast[:].rearrange("p (c f) -> p c f", c=NW), in_=u_ap)
                nc.gpsimd.collective_compute(
                    "AllGather",
                    mybir.AluOpType.bypass,
                    replica_groups=[list(range(NCORES))],
                    ins=[vm_slice[:, :]],
                    outs=[vm_full[0:N_ATOMS, :]],
                )

                # ---- edge phase
                for w in range(NW):
                    ntw = nts[w]
                    idxt = idxw.tile([128, nt_u * 3], I32, tag="idxt")
                    nc.sync.dma_start(
                        out=idxt[:, 0:3 * ntw],
                        in_=pack_d[w * 128:(w + 1) * 128, 0:3 * ntw],
                    )
                    pw = pwin.tile([128, H], F32, space="PSUM")
                    uwin = ucast[:, w * H:(w + 1) * H]
                    for t in range(ntw):
                        if t % 2 == 0:
                            vmg2 = edge.tile([128, 4 * H], BF16, tag="vmg2")
                        vmg = vmg2[:, (t % 2) * 2 * H:((t % 2) + 1) * 2 * H]
                        nc.gpsimd.indirect_dma_start(
                            out=vmg, out_offset=None, in_=vm_full[:, :],
                            in_offset=bass.IndirectOffsetOnAxis(
                                ap=idxt[:, 3 * t:3 * t + 1], axis=0),
                        )
                        st = edge.tile([128, 128], BF16)
                        nc.vector.tensor_scalar(
                            out=st[:], in0=iot[:],
                            scalar1=idxt[:, 3 * t + 2:3 * t + 3].bitcast(F32), scalar2=None,
                            op0=mybir.AluOpType.is_equal,
                        )
                        # U_exp = S @ U_window via PE: transpose S then matmul
                        stp = pst.tile([128, 128], BF16, space="PSUM")
                        nc.tensor.transpose(out=stp[:], in_=st[:], identity=identb[:])
                        stT = edge.tile([128, 128], BF16)
                        nc.scalar.copy(out=stT[:], in_=stp[:])
                        uep = puex.tile([128, H], F32, space="PSUM")
                        nc.tensor.matmul(uep[:], lhsT=stT[:], rhs=uwin,
                                         start=True, stop=True)
                        gp = edge.tile([128, H], F32)
                        nc.vector.tensor_add(out=gp[:], in0=vmg[:, 0:H], in1=uep[:])
                        nc.scalar.activation(
                            out=gp[:], in_=gp[:],
                            func=mybir.ActivationFunctionType.Sigmoid,
                        )
                        msg = edge.tile([128, H], BF16)
                        nc.vector.tensor_mul(out=msg[:], in0=gp[:], in1=vmg[:, H:2 * H])
                        nc.tensor.matmul(
                            pw[:], lhsT=st[:], rhs=msg[:],
                            start=(t == 0), stop=(t == ntw - 1),
                        )
                    # x_new = relu(agg + M)
                    xw = xnew[:, w * H:(w + 1) * H]
                    nc.vector.tensor_add(
                        out=xw, in0=pw[:],
                        in1=uvm[:, w * 3 * H + H:w * 3 * H + 2 * H],
                    )
                    nc.vector.tensor_relu(out=xw, in_=xw)

                if l < N_LAYERS - 1:
                    # transpose x_new back into xTb for the next node phase
                    for cch in range(NW):
                        pt = ptr.tile([64, 128], F32, space="PSUM")
                        nc.tensor.transpose(
                            out=pt[:],
                            in_=xnew[:, cch * H:(cch + 1) * H],
                            identity=ident[:],
                        )
                        nc.vector.tensor_copy(
                            out=xTb[0:H, cch * 128:(cch + 1) * 128], in_=pt[:]
                        )

            # ---- masked mean-pool partial
            pp = ppool.tile([1, H], F32, space="PSUM")
            for w in range(NW):
                nc.tensor.matmul(
                    pp[:], lhsT=pm[:, w:w + 1], rhs=xnew[:, w * H:(w + 1) * H],
                    start=(w == 0), stop=(w == NW - 1),
                )
            pout = small.tile([1, H], F32)
            nc.vector.tensor_copy(out=pout[:], in_=pp[:])
            nc.sync.dma_start(out=pooled_d[:, :], in_=pout[:])

    nc.compile()
    return nc


def kernel(**inputs) -> np.ndarray:
    packs, wcat, x0T, pm, iot, nt_u, nts = _prep(inputs)
    nc = _build(nt_u, nts)
    in_maps = [
        {"pack": packs[c], "wcat": wcat, "x0t": x0T[c], "pm": pm, "iota": iot}
        for c in range(NCORES)
    ]
    res = bass_utils.run_bass_kernel_spmd(nc, in_maps, core_ids=list(range(NCORES)))
    global LAST_RESULTS
    LAST_RESULTS = res
    total = np.zeros(H, np.float64)
    for c in range(NCORES):
        total += res.results[c]["pooled"].reshape(H).astype(np.float64)
    pooled = (total / N_ATOMS).astype(np.float32)
    h = np.maximum(pooled @ np.asarray(inputs["W_out1"], np.float32)
                   + np.asarray(inputs["b_out1"], np.float32), 0.0)
    out = h @ np.asarray(inputs["W_out2"], np.float32) + np.asarray(
        inputs["b_out2"], np.float32)
    return out.astype(np.float32)


# revision 21
# speedup vs baseline: 1.0199x; 1.0199x over previous
"""CrystalGraphEncoder Trainium2 kernel (8 NeuronCores, SPMD).

Sharding: nodes split into 8 contiguous row-slices; each core owns edges whose
destination row falls in its slice (sorted by local row). Per layer, each core
computes per-node tables [V|M|U] = x @ [Wg2|Wl|Wg1] + biases for its slice,
the V|M part is AllGathered into a full replicated [N,128] table, then the
edge phase gathers VM[col] and U[row] per 128-edge tile (indirect DMA),
computes msg = sigmoid(U+V) * M, and segment-sums via a one-hot selection
matmul into PSUM per 128-row window. x <- relu(agg + M). Mean-pool partials
are returned per core; the tiny MLP head runs on host.
"""
import sys
import os

sys.path.insert(0, "/opt/trn_rl_repo")

import numpy as np

import concourse.bass as bass
import concourse.mybir as mybir
import concourse.tile as tile
from concourse import bacc
from concourse import bass_utils

# bass_utils imports antenv.axon_hooks when BASS_TRACE is set; provide a
# graceful stub if the image's antenv lacks that module.
try:
    import antenv.axon_hooks  # noqa: F401
except Exception:
    import types as _types
    import antenv as _antenv

    _hooks = _types.ModuleType("antenv.axon_hooks")
    _hooks._HOOK = None
    _hooks.set_axon_ntff_profile_hook = lambda h: setattr(_hooks, "_HOOK", h)
    _hooks.get_axon_ntff_profile_hook = lambda: _hooks._HOOK
    sys.modules["antenv.axon_hooks"] = _hooks
    _antenv.axon_hooks = _hooks

F32 = mybir.dt.float32
BF16 = mybir.dt.bfloat16
I32 = mybir.dt.int32

N_ATOMS = int(os.environ.get("GNN_N", 100000))
N_EDGES = 1000000
H = 64
OUT_DIM = 128
N_LAYERS = 3
NCORES = 8
S = N_ATOMS // NCORES          # 12500 rows per core
NW = (S + 127) // 128          # 98 windows per core
SPAD = NW * 128                # 12544 padded rows
LASTW = S - (NW - 1) * 128     # 84 valid rows in final window
PADCOL = N_ATOMS               # zero row of the VM table


def _prep(inputs):
    """Host-side prep: embedding lookup, per-core edge scheduling, weights."""
    x0 = np.asarray(inputs["emb_table"], np.float32)[np.asarray(inputs["atomic_numbers"])]
    edge = np.asarray(inputs["edge_index"])
    row = edge[0].astype(np.int64)
    col = edge[1].astype(np.int64)

    core_of = row // S
    percore = []
    cntmax = np.zeros(NW, np.int64)
    for c in range(NCORES):
        m = core_of == c
        lr = (row[m] - c * S).astype(np.int32)
        cc = col[m].astype(np.int32)
        o = np.argsort(lr, kind="stable")
        lr, cc = lr[o], cc[o]
        w = lr >> 7
        cnt = np.bincount(w, minlength=NW)
        cntmax = np.maximum(cntmax, cnt)
        percore.append((lr, cc, w, cnt))
    nts = np.maximum(np.ceil(cntmax / 128).astype(np.int64), 1)
    nt_u = int(nts.max())

    # Build uniform [NW, 128, nt_u] (flattened to [SPAD, nt_u*3]) idx arrays.
    packs = []
    for c in range(NCORES):
        lr, cc, w, cnt = percore[c]
        colidx = np.full((NW, nt_u * 128), PADCOL, np.int32)
        uidx = np.zeros((NW, nt_u * 128), np.int32)
        rrel = np.zeros((NW, nt_u * 128), np.float32)
        start = 0
        for wi in range(NW):
            n = int(cnt[wi])
            sl = slice(start, start + n)
            colidx[wi, :n] = cc[sl]
            uidx[wi, :n] = lr[sl]
            rrel[wi, :n] = (lr[sl] & 127).astype(np.float32)
            start += n
        # slot e = t*128 + p  ->  [NW, nt_u, 128] -> [NW, 128, nt_u]
        colidx = colidx.reshape(NW, nt_u, 128).transpose(0, 2, 1)
        uidx = uidx.reshape(NW, nt_u, 128).transpose(0, 2, 1)
        rrel = rrel.reshape(NW, nt_u, 128).transpose(0, 2, 1)
        pack = np.empty((NW, 128, nt_u, 3), np.int32)
        pack[..., 0] = colidx
        pack[..., 1] = uidx
        pack[..., 2] = rrel.view(np.int32)
        packs.append(pack.reshape(SPAD, nt_u * 3))

    # weights: wcat [L, 65, 192] = rows 0..63 [Wg2|Wl|Wg1], row 64 biases [0|bl|bg]
    Wg = np.asarray(inputs["W_gate"], np.float32)   # [L, 2H, H]
    Wl = np.asarray(inputs["W_lin"], np.float32)    # [L, H, H]
    bl = np.asarray(inputs["b_lin"], np.float32)    # [L, H]
    bg = np.asarray(inputs["b_gate"], np.float32)   # [L, H]
    wcat = np.zeros((N_LAYERS, 65, 3 * H), np.float32)
    for l in range(N_LAYERS):
        wcat[l, :H, 0:H] = Wg[l, H:]       # V = x @ Wg2
        wcat[l, :H, H:2 * H] = Wl[l]       # M = x @ Wl
        wcat[l, :H, 2 * H:] = Wg[l, :H]    # U = x @ Wg1
        wcat[l, 64, H:2 * H] = bl[l]
        wcat[l, 64, 2 * H:] = bg[l]

    # x0T slices [64, SPAD] f32, zero-padded
    x0T = []
    for c in range(NCORES):
        xs = np.zeros((H, SPAD), np.float32)
        xs[:, :S] = x0[c * S:(c + 1) * S].T
        x0T.append(np.ascontiguousarray(xs))

    # pooling mask [128, NW] f32
    pm = np.ones((128, NW), np.float32)
    pm[LASTW:, NW - 1] = 0.0

    # iota row replicated [128, 128] f32
    iot = np.broadcast_to(np.arange(128, dtype=np.float32), (128, 128)).copy()

    return packs, wcat, x0T, pm, iot, nt_u, [int(v) for v in nts]


def _build(nt_u, nts):
    nc = bacc.Bacc("TRN2", target_bir_lowering=False, debug=False,
                   num_devices=NCORES)
    pack_d = nc.dram_tensor("pack", [SPAD, nt_u * 3], I32, kind="ExternalInput")
    wcat_d = nc.dram_tensor("wcat", [N_LAYERS, 65, 3 * H], F32, kind="ExternalInput")
    x0t_d = nc.dram_tensor("x0t", [H, SPAD], F32, kind="ExternalInput")
    pm_d = nc.dram_tensor("pm", [128, NW], F32, kind="ExternalInput")
    iota_d = nc.dram_tensor("iota", [128, 128], F32, kind="ExternalInput")
    pooled_d = nc.dram_tensor("pooled", [1, H], F32, kind="ExternalOutput")

    vm_slice = nc.dram_tensor("vm_slice", [S, 2 * H], BF16, kind="Internal")
    vm_full = nc.dram_tensor("vm_full", [N_ATOMS + 1, 2 * H], BF16,
                             kind="Internal", addr_space="Shared")
    u_slice = nc.dram_tensor("u_slice", [S, H], F32, kind="Internal")

    with tile.TileContext(nc) as tc:
        with (
            tc.tile_pool(name="persist", bufs=1) as persist,
            tc.tile_pool(name="stage", bufs=1) as stage,
            tc.tile_pool(name="idxw", bufs=4) as idxw,
            tc.tile_pool(name="edge", bufs=8) as edge,
            tc.tile_pool(name="small", bufs=2) as small,
            tc.tile_pool(name="pnode", bufs=1, space="PSUM") as pnode,
            tc.tile_pool(name="pwin", bufs=3, space="PSUM") as pwin,
            tc.tile_pool(name="ptr", bufs=1, space="PSUM") as ptr,
            tc.tile_pool(name="ppool", bufs=1, space="PSUM") as ppool,
            tc.tile_pool(name="pst", bufs=1, space="PSUM") as pst,
            tc.tile_pool(name="puex", bufs=1, space="PSUM") as puex,
        ):
            xTb = persist.tile([65, SPAD], F32)       # row 64 = ones
            uvm = stage.tile([128, NW * 3 * H], F32)  # [V|M|U] per chunk
            xnew = stage.tile([128, NW * H], F32)
            vmcast = stage.tile([128, NW * 2 * H], BF16)
            ucast = stage.tile([128, NW * H], BF16)
            iot = persist.tile([128, 128], F32)
            pm = persist.tile([128, NW], F32)
            wct = persist.tile([65, N_LAYERS * 3 * H], F32)
            ident = persist.tile([128, 128], F32)
            identb = persist.tile([128, 128], BF16)

            nc.sync.dma_start(out=iot[:], in_=iota_d[:, :])
            nc.sync.dma_start(out=pm[:], in_=pm_d[:, :])
            nc.sync.dma_start(
                out=wct[:].rearrange("p (l f) -> p l f", l=N_LAYERS),
                in_=wcat_d[:, :, :].rearrange("l p f -> p l f"),
            )
            nc.sync.dma_start(out=xTb[0:H, :], in_=x0t_d[:, :])
            nc.vector.memset(xTb[64:65, :], 1.0)
            # identity for PE transpose: ident[p, j] = (j == p)
            iotp = persist.tile([128, 1], F32)
            nc.gpsimd.iota(iotp[:], pattern=[[1, 1]], base=0, channel_multiplier=1,
                           allow_small_or_imprecise_dtypes=True)
            nc.vector.tensor_scalar(
                out=ident[:], in0=iot[:], scalar1=iotp[:, 0:1], scalar2=None,
                op0=mybir.AluOpType.is_equal,
            )
            nc.vector.tensor_copy(out=identb[:], in_=ident[:])
            # zero the pad row of vm_full once
            zrow = small.tile([1, 2 * H], BF16)
            nc.vector.memset(zrow[:], 0.0)
            nc.sync.dma_start(out=vm_full[N_ATOMS:N_ATOMS + 1, :], in_=zrow[:])

            for l in range(N_LAYERS):
                # ---- node phase: [V|M|U] = xTb.T @ wcat[l] per 128-row chunk
                for cch in range(NW):
                    ps = pnode.tile([128, 3 * H], F32, space="PSUM")
                    nc.tensor.matmul(
                        ps[:],
                        lhsT=xTb[:, cch * 128:(cch + 1) * 128],
                        rhs=wct[:, l * 3 * H:(l + 1) * 3 * H],
                        start=True, stop=True,
                    )
                    nc.scalar.copy(
                        out=uvm[:, cch * 3 * H:(cch + 1) * 3 * H], in_=ps[:]
                    )
                # table writes (rows < S only), cast to bf16 first
                vm_ap = uvm[:].rearrange("p (c f) -> p c f", c=NW)[:, :, 0:2 * H]
                vmc_ap = vmcast[:].rearrange("p (c f) -> p c f", c=NW)
                nc.vector.tensor_copy(out=vmc_ap[:, :, :], in_=vm_ap)
                nc.sync.dma_start(
                    out=vm_slice[0:(NW - 1) * 128, :].rearrange(
                        "(c p) f -> p c f", p=128),
                    in_=vmc_ap[:, 0:NW - 1, :],
                )
                nc.sync.dma_start(
                    out=vm_slice[(NW - 1) * 128:S, :],
                    in_=vmc_ap[0:LASTW, NW - 1, :],
                )
                u_ap = uvm[:].rearrange("p (c f) -> p c f", c=NW)[:, :, 2 * H:3 * H]
                nc.vector.tensor_copy(
                    out=ucast[:].rearrange("p (c f) -> p c f", c=NW), in_=u_ap)
                nc.gpsimd.collective_compute(
                    "AllGather",
                    mybir.AluOpType.bypass,
                    replica_groups=[list(range(NCORES))],
                    ins=[vm_slice[:, :]],
                    outs=[vm_full[0:N_ATOMS, :]],
                )

                # ---- edge phase
                for w in range(NW):
                    ntw = nts[w]
                    idxt = idxw.tile([128, nt_u * 3], I32, tag="idxt")
                    nc.sync.dma_start(
                        out=idxt[:, 0:3 * ntw],
                        in_=pack_d[w * 128:(w + 1) * 128, 0:3 * ntw],
                    )
                    pw = pwin.tile([128, H], F32, space="PSUM")
                    uwin = ucast[:, w * H:(w + 1) * H]
                    for t in range(ntw):
                        vmg = edge.tile([128, 2 * H], BF16)
                        nc.gpsimd.indirect_dma_start(
                            out=vmg[:], out_offset=None, in_=vm_full[:, :],
                            in_offset=bass.IndirectOffsetOnAxis(
                                ap=idxt[:, 3 * t:3 * t + 1], axis=0),
                        )
                        st = edge.tile([128, 128], BF16)
                        nc.vector.tensor_scalar(
                            out=st[:], in0=iot[:],
                            scalar1=idxt[:, 3 * t + 2:3 * t + 3].bitcast(F32), scalar2=None,
                            op0=mybir.AluOpType.is_equal,
                        )
                        # U_exp = S @ U_window via PE: transpose S then matmul
                        stp = pst.tile([128, 128], BF16, space="PSUM")
                        nc.tensor.transpose(out=stp[:], in_=st[:], identity=identb[:])
                        stT = edge.tile([128, 128], BF16)
                        nc.scalar.copy(out=stT[:], in_=stp[:])
                        uep = puex.tile([128, H], F32, space="PSUM")
                        nc.tensor.matmul(uep[:], lhsT=stT[:], rhs=uwin,
                                         start=True, stop=True)
                        gp = edge.tile([128, H], F32)
                        nc.vector.tensor_add(out=gp[:], in0=vmg[:, 0:H], in1=uep[:])
                        nc.scalar.activation(
                            out=gp[:], in_=gp[:],
                            func=mybir.ActivationFunctionType.Sigmoid,
                        )
                        msg = edge.tile([128, H], BF16)
                        nc.vector.tensor_mul(out=msg[:], in0=gp[:], in1=vmg[:, H:2 * H])
                        nc.tensor.matmul(
                            pw[:], lhsT=st[:], rhs=msg[:],
                            start=(t == 0), stop=(t == ntw - 1),
                        )
                    # x_new = relu(agg + M)
                    xw = xnew[:, w * H:(w + 1) * H]
                    nc.vector.tensor_add(
                        out=xw, in0=pw[:],
                        in1=uvm[:, w * 3 * H + H:w * 3 * H + 2 * H],
                    )
                    nc.vector.tensor_relu(out=xw, in_=xw)

                if l < N_LAYERS - 1:
                    # transpose x_new back into xTb for the next node phase
                    for cch in range(NW):
                        pt = ptr.tile([64, 128], F32, space="PSUM")
                        nc.tensor.transpose(
                            out=pt[:],
                            in_=xnew[:, cch * H:(cch + 1) * H],
                            identity=ident[:],
                        )
                        nc.vector.tensor_copy(
                            out=xTb[0:H, cch * 128:(cch + 1) * 128], in_=pt[:]
                        )

            # ---- masked mean-pool partial
            pp = ppool.tile([1, H], F32, space="PSUM")
            for w in range(NW):
                nc.tensor.matmul(
                    pp[:], lhsT=pm[:, w:w + 1], rhs=xnew[:, w * H:(w + 1) * H],
                    start=(w == 0), stop=(w == NW - 1),
                )
            pout = small.tile([1, H], F32)
            nc.vector.tensor_copy(out=pout[:], in_=pp[:])
            nc.sync.dma_start(out=pooled_d[:, :], in_=pout[:])

    nc.compile()
    return nc


def kernel(**inputs) -> np.ndarray:
    packs, wcat, x0T, pm, iot, nt_u, nts = _prep(inputs)
    nc = _build(nt_u, nts)
    in_maps = [
        {"pack": packs[c], "wcat": wcat, "x0t": x0T[c], "pm": pm, "iota": iot}
        for c in range(NCORES)
    ]
    res = bass_utils.run_bass_kernel_spmd(nc, in_maps, core_ids=list(range(NCORES)))
    global LAST_RESULTS
    LAST_RESULTS = res
    total = np.zeros(H, np.float64)
    for c in range(NCORES):
        total += res.results[c]["pooled"].reshape(H).astype(np.float64)
    pooled = (total / N_ATOMS).astype(np.float32)
    h = np.maximum(pooled @ np.asarray(inputs["W_out1"], np.float32)
                   + np.asarray(inputs["b_out1"], np.float32), 0.0)
    out = h @ np.asarray(inputs["W_out2"], np.float32) + np.asarray(
        inputs["b_out2"], np.float32)
    return out.astype(np.float32)
